# revision 17
# baseline (speedup 1.0000x reference)
"""BitNet MLP (ternary gate/up GEMM + silu*up + Hadamard + act-quant + down GEMM)
on 8 Trainium2 NeuronCores — v4.

Token-data-parallel (T=1024 tokens/core). v3 -> v4 (trace-driven):
  - butterflies are bit-exact via a spare-chunk ping-pong (2 tensor_tensor
    ops per pair, physical-slot indirection) instead of the lossy 2a-(a-b)
    form; still run in-flight during GEMM1, burst-smoothed,
  - per-token absmax of the intermediate is accumulated CONTIGUOUSLY
    (abs_max accumulate after each stage-6 pair) instead of one giant
    strided reduce,
  - intermediate act-quant is exact f32 (PSUM-staged MAGIC rint, matching
    v2 numerics) emitted chunk-major so GEMM2's hc=0 pipelines behind the
    quant wavefront,
  - GEMM1 emits all four matmul groups before the epilogues and defers the
    tt1 H128 matmul into the next chunk's PE stream (no FIFO stall on the
    epilogue chain); H128 runs on float32r (1 cyc/row),
  - GEMM2 is hc-outer with both token halves interleaved per wd piece so
    w_down is DMA'd + ternarized exactly once; ternarize add on DVE.
"""

import sys

sys.path.insert(0, "/opt/trn_rl_repo")

import numpy as np

import concourse.bass as bass
import concourse.mybir as mybir
import concourse.tile as tile
from concourse import bacc, bass_isa

F32 = mybir.dt.float32
F32R = mybir.dt.float32r
BF16 = mybir.dt.bfloat16
AX = mybir.AxisListType.X
OP = mybir.AluOpType
ACT_FN = mybir.ActivationFunctionType

MAGIC = 12582912.0  # 1.5*2^23: (x + MAGIC) - MAGIC == rint(x) in f32
EPS = 1e-5
QCLIP = 127.4375


def hadamard128():
    h = np.array([[1.0]], dtype=np.float32)
    while h.shape[0] < 128:
        h = np.block([[h, h], [h, -h]]).astype(np.float32)
    return h


def build_program(T, H, I, n_cores):
    from concourse.masks import make_identity

    P = 128
    HC = H // P                    # 16 h-chunks
    C = I // P                     # 64 i-chunks (fwht H64 factor)
    NT = min(512, T)
    TTN = T // NT                  # 2 token halves
    n_tb = T // P                  # 8 token 128-blocks
    L = int(np.log2(C))            # 6 butterfly stages over the chunk index
    assert 2 ** L == C and T % P == 0 and H % 1024 == 0 and I % P == 0
    inv_sqrt_i = float(1.0 / np.sqrt(I))
    wcount = float(I) * float(H)

    nc = bacc.Bacc("TRN2", target_bir_lowering=False, num_devices=n_cores)

    x_d = nc.dram_tensor("x_s", [T, H], F32, kind="ExternalInput")
    wgt_d = nc.dram_tensor("wgt", [H, I], F32, kind="ExternalInput")
    wut_d = nc.dram_tensor("wut", [H, I], F32, kind="ExternalInput")
    wdt_d = nc.dram_tensor("wdt", [I, H], F32, kind="ExternalInput")
    wgs_d = nc.dram_tensor("wg_s", [H // n_cores, I], F32, kind="ExternalInput")
    wus_d = nc.dram_tensor("wu_s", [H // n_cores, I], F32, kind="ExternalInput")
    wds_d = nc.dram_tensor("wd_s", [I // n_cores, H], F32, kind="ExternalInput")
    hm_d = nc.dram_tensor("hmat", [P, P], F32, kind="ExternalInput")
    out_d = nc.dram_tensor("out_s", [T, H], F32, kind="ExternalOutput")

    cc_in = nc.dram_tensor("cc_in", [1, 4], F32)
    cc_out = nc.dram_tensor("cc_out", [1, 4], F32, addr_space="Shared")

    # DRAM views for transposed-tile loads: [p, q, i] with h = q*128 + p
    wgt_v = wgt_d.ap().rearrange("(q p) i -> p q i", p=P)
    wut_v = wut_d.ap().rearrange("(q p) i -> p q i", p=P)
    wdt_v = wdt_d.ap().rearrange("(c p) h -> p c h", p=P)

    with tile.TileContext(nc) as tc:
        with (
            tc.tile_pool(name="consts", bufs=1) as consts,
            tc.tile_pool(name="wpipe", bufs=2) as wpipe,    # f32 stage [P,8,128]
            tc.tile_pool(name="wqp", bufs=1) as wqp,        # ternarized weights
            tc.tile_pool(name="xq", bufs=1) as xqp,         # xqT persistent
            tc.tile_pool(name="ip", bufs=1) as ip,          # interm
            tc.tile_pool(name="s2k", bufs=1) as s2k,        # scratch
            tc.tile_pool(name="sc", bufs=1) as sc,          # scales/rows
            tc.tile_pool(name="ps", bufs=1, space="PSUM") as psp,
        ):
            # ---------------- constants
            hmtmp = wpipe.tile([P, 8, P], F32, tag="wf32", name="hmtmp")
            nc.sync.dma_start(hmtmp[:, 0, :], hm_d.ap())
            ident_f = consts.tile([P, P], F32, tag="ident_f")
            make_identity(nc, ident_f[:])
            ident_b = consts.tile([P, P], BF16, tag="ident_b")
            nc.vector.tensor_copy(ident_b[:], ident_f[:])
            hmat_r = consts.tile([P, P], F32R, tag="hmat_r")
            nc.vector.tensor_copy(hmat_r[:], hmtmp[:, 0, :])
            magicB = consts.tile([P, 1], F32, tag="magicB")
            nc.vector.memset(magicB[:], MAGIC)
            nmagicB = consts.tile([P, 1], F32, tag="nmagicB")
            nc.vector.memset(nmagicB[:], -MAGIC)
            ones1 = consts.tile([1, P], F32, tag="ones1")
            nc.vector.memset(ones1[:], 1.0)
            halfn = consts.tile([P, 1], F32, tag="halfn")
            nc.vector.memset(halfn[:], -0.5)
            halfp = consts.tile([P, 1], F32, tag="halfp")
            nc.vector.memset(halfp[:], 0.5)
            m1 = sc.tile([P, T], BF16, tag="m1")
            nc.vector.memset(m1[:], 0.0)

            # ---------------- x: act-quant + transpose into xqT (bf16 ints)
            xqt = xqp.tile([P, HC, T], BF16, tag="xqt")
            am_row = sc.tile([1, T], F32, tag="am_row")
            for tb in range(n_tb):
                xt = [wpipe.tile([P, 8, P], F32, tag="wf32", name=f"x{tb}_{q}")
                      for q in range(2)]
                for q in range(2):
                    nc.sync.dma_start(
                        xt[q][:].rearrange("p a b -> p (a b)"),
                        x_d.ap()[tb * P:(tb + 1) * P,
                                 q * 1024:(q + 1) * 1024])
                am2 = sc.tile([P, 2], F32, tag="am2")
                for q in range(2):
                    nc.vector.tensor_reduce(
                        out=am2[:, q:q + 1],
                        in_=xt[q][:].rearrange("p a b -> p (a b)"),
                        op=OP.max, axis=AX, apply_absolute_value=True)
                amc = sc.tile([P, 1], F32, tag="amc")
                nc.vector.tensor_reduce(out=amc[:], in_=am2[:], op=OP.max,
                                        axis=AX)
                nc.vector.tensor_scalar(amc[:], amc[:], EPS, None, OP.max)
                sx = sc.tile([P, 1], F32, tag="sx")
                nc.vector.reciprocal(sx[:], amc[:])
                nc.vector.tensor_scalar(sx[:], sx[:], 128.0, None, OP.mult)
                for q in range(2):
                    f = xt[q][:].rearrange("p a b -> p (a b)")
                    nc.scalar.activation(f, f, ACT_FN.Identity,
                                         bias=magicB[:], scale=sx[:])
                    xqb = s2k.tile([P, 1024], BF16, tag="ta",
                                   name=f"xqb{tb}_{q}")
                    nc.vector.tensor_scalar(xqb[:], f, -MAGIC, 127.0,
                                            OP.add, OP.min)
                    for k in range(8):
                        hc = q * 8 + k
                        pt = psp.tile([P, P], BF16, tag="tp", bufs=2,
                                      name=f"pt{tb}_{hc}")
                        nc.tensor.transpose(pt[:], xqb[:, k * P:(k + 1) * P],
                                            ident_b[:])
                        nc.scalar.copy(xqt[:, hc, tb * P:(tb + 1) * P], pt[:])
                pr = psp.tile([P, P], F32, tag="tp", bufs=2, name=f"pr{tb}")
                nc.tensor.transpose(pr[:1, :], amc[:], ident_f[:])
                nc.scalar.copy(am_row[:, tb * P:(tb + 1) * P], pr[:1, :])

            # ---------------- weight-scale pass (shard |w| sums + AllReduce)
            def shard_abs_sum(src_d, rows, cols, tag):
                ntr, ntc = rows // P, cols // 1024
                acc = sc.tile([P, ntr * ntc], F32, tag=f"acc_{tag}")
                for r in range(ntr):
                    for q in range(ntc):
                        t = wpipe.tile([P, 8, P], F32, tag="wf32",
                                       name=f"as_{tag}_{r}_{q}")
                        nc.sync.dma_start(
                            t[:].rearrange("p a b -> p (a b)"),
                            src_d.ap()[r * P:(r + 1) * P,
                                       q * 1024:(q + 1) * 1024])
                        nc.vector.tensor_reduce(
                            out=acc[:, r * ntc + q:r * ntc + q + 1],
                            in_=t[:].rearrange("p a b -> p (a b)"),
                            op=OP.add, axis=AX, apply_absolute_value=True)
                tot = sc.tile([P, 1], F32, tag=f"tot_{tag}")
                nc.vector.tensor_reduce(out=tot[:], in_=acc[:], op=OP.add,
                                        axis=AX)
                red = sc.tile([P, 1], F32, tag=f"red_{tag}")
                nc.gpsimd.partition_all_reduce(
                    red[:], tot[:], channels=P, reduce_op=bass_isa.ReduceOp.add)
                return red

            red_g = shard_abs_sum(wgs_d, H // n_cores, I, "g")
            red_u = shard_abs_sum(wus_d, H // n_cores, I, "u")
            red_d = shard_abs_sum(wds_d, I // n_cores, H, "d")

            ccin_sb = sc.tile([1, 4], F32, tag="ccin")
            nc.vector.memset(ccin_sb[:], 0.0)
            nc.vector.tensor_copy(ccin_sb[:, 0:1], red_g[0:1, :])
            nc.vector.tensor_copy(ccin_sb[:, 1:2], red_u[0:1, :])
            nc.vector.tensor_copy(ccin_sb[:, 2:3], red_d[0:1, :])
            nc.sync.dma_start(cc_in.ap(), ccin_sb[:])
            nc.gpsimd.collective_compute(
                "AllReduce", OP.add, ins=[cc_in.ap()], outs=[cc_out.ap()],
                replica_groups=[list(range(n_cores))])
            sums_sb = sc.tile([1, 4], F32, tag="sums")
            nc.sync.dma_start(sums_sb[:], cc_out.ap())

            # ---------------- finalize weight scales (waits on AllReduce)
            wm_row = sc.tile([1, 4], F32, tag="wm_row")
            nc.vector.tensor_scalar(wm_row[:], sums_sb[:], 1.0 / wcount, EPS,
                                    OP.mult, OP.max)
            ws_row = sc.tile([1, 4], F32, tag="ws_row")
            nc.vector.reciprocal(ws_row[:], wm_row[:])
            wsB = sc.tile([P, 4], F32, tag="wsB")
            nc.gpsimd.partition_broadcast(wsB[:], ws_row[:])

            # bcast = per-token gate dequant (am * wm_g / 256), all partitions
            sg = sc.tile([1, 1], F32, tag="sg")
            nc.vector.tensor_scalar(sg[:], wm_row[:, 0:1], 1.0 / 256.0, None,
                                    OP.mult)
            sgB = sc.tile([P, 1], F32, tag="sgB")
            nc.gpsimd.partition_broadcast(sgB[:], sg[:])
            bcast = sc.tile([P, T], F32, tag="bcast")
            nc.gpsimd.partition_broadcast(bcast[:], am_row[:])
            nc.vector.tensor_scalar(bcast[:], bcast[:], sgB[:], None, OP.mult)

            # ---------------- ternarize helper: f32 tile -> bf16 2*{-1,0,1}
            # 2*clip(rint(w*ws),-1,1) = Sign(w*ws-0.5) + Sign(w*ws+0.5)
            _tz = [0]
            def ternarize(wt_f, dst_b, ws_ap, add_eng):
                _tz[0] += 1
                ta = s2k.tile([P, 1024], BF16, tag="ta", name=f"ta{_tz[0]}")
                nc.scalar.activation(dst_b, wt_f, ACT_FN.Sign,
                                     bias=halfn[:], scale=ws_ap)
                nc.scalar.activation(ta[:], wt_f, ACT_FN.Sign,
                                     bias=halfp[:], scale=ws_ap)
                add_eng.tensor_tensor(dst_b, dst_b, ta[:], OP.add)

            # ---------------- in-flight H64 butterfly, bit-exact via a
            # spare physical chunk slot: s = a+b -> spare ; b = a-b in place;
            # the old a-slot becomes the new spare.
            interm = ip.tile([P, C + 1, T], BF16, tag="interm")
            loc = list(range(C))
            spare = [C]

            _ab = [0]
            def butterfly(stage, a_idx, b_idx):
                pa, pb, sp = loc[a_idx], loc[b_idx], spare[0]
                A = interm[:, pa, :]
                B = interm[:, pb, :]
                S_ = interm[:, sp, :]
                # stage-6 pairs are mutually independent: give some to gpsimd
                eng = (nc.gpsimd if (stage == L and a_idx % 4 == 3)
                       else nc.vector)
                eng.tensor_tensor(S_, A, B, OP.add)
                eng.tensor_tensor(B, A, B, OP.subtract)
                loc[a_idx] = sp
                spare[0] = pa
                if stage == L:
                    # accumulate per-(partition,token) absmax: |chunk| on the
                    # (otherwise idle) ACT engine, single max chain on DVE
                    for pidx in (loc[a_idx], loc[b_idx]):
                        _ab[0] += 1
                        ab = s2k.tile([P, T], BF16, tag="ab", bufs=2,
                                      name=f"ab{_ab[0]}")
                        nc.scalar.activation(ab[:], interm[:, pidx, :],
                                             ACT_FN.Abs)
                        nc.vector.tensor_tensor(m1[:], m1[:], ab[:], OP.max)

            pending = []  # butterfly pairs not yet emitted (burst smoothing)

            # ---------------- GEMM1 + epilogue (H128 fused) -> interm bf16
            def epi_dve(ic, tt, ps_g, ps_u):
                ts = slice(tt * NT, (tt + 1) * NT)
                g1 = s2k.tile([P, NT], F32R, tag="g1", bufs=2,
                              name=f"g1_{ic}{tt}")
                nc.vector.tensor_tensor(g1[:], ps_g[:], bcast[:, ts],
                                        OP.mult)
                nc.scalar.activation(g1[:], g1[:], ACT_FN.Silu)
                nc.vector.tensor_tensor(g1[:], g1[:], ps_u[:], OP.mult)
                return g1

            def epi_h128(ic, tt, g1):
                ts = slice(tt * NT, (tt + 1) * NT)
                psH = psp.tile([P, NT], F32, tag="mm", bufs=6,
                               name=f"psh{ic}_{tt}")
                nc.tensor.matmul(psH[:], hmat_r[:], g1[:],
                                 start=True, stop=True)
                nc.scalar.copy(interm[:, ic, ts], psH[:])

            pend_h = [None]
            for ic in range(C):
                wq = []
                for mi, w_v in ((0, wgt_v), (1, wut_v)):
                    w = wqp.tile([P, HC, P], BF16, tag=f"wq{mi}",
                                 bufs=(2 if mi == 0 else 1),
                                 name=f"wq{mi}_{ic}")
                    wq.append(w)
                    for half in range(2):
                        wt = wpipe.tile([P, 8, P], F32, tag="wf32",
                                        name=f"w{ic}_{mi}_{half}")
                        nc.sync.dma_start(
                            wt[:],
                            w_v[:, half * 8:(half + 1) * 8,
                                ic * P:(ic + 1) * P])
                        ternarize(
                            wt[:].rearrange("p a b -> p (a b)"),
                            w[:, half * 8:(half + 1) * 8, :].rearrange(
                                "p a b -> p (a b)"),
                            wsB[:, mi:mi + 1], nc.vector)
                ps = []
                for tt in range(TTN):
                    ts = slice(tt * NT, (tt + 1) * NT)
                    ps_g = psp.tile([P, NT], F32, tag="mm", bufs=6,
                                    name=f"psg{ic}_{tt}")
                    ps_u = psp.tile([P, NT], F32, tag="mm", bufs=6,
                                    name=f"psu{ic}_{tt}")
                    ps.append((ps_g, ps_u))
                    for hc in range(HC):
                        nc.tensor.matmul(ps_g[:], wq[0][:, hc, :],
                                         xqt[:, hc, ts],
                                         start=(hc == 0), stop=(hc == HC - 1))
                    for hc in range(HC):
                        nc.tensor.matmul(ps_u[:], wq[1][:, hc, :],
                                         xqt[:, hc, ts],
                                         start=(hc == 0), stop=(hc == HC - 1))
                    if tt == 0:
                        # H128 of the previous chunk's tt1 (deferred: its
                        # product is ready by now; 2 mm groups of slack)
                        if pend_h[0] is not None:
                            epi_h128(*pend_h[0])
                            pend_h[0] = None
                        g1_0 = epi_dve(ic, 0, ps_g, ps_u)
                    else:
                        epi_h128(ic, 0, g1_0)
                        g1_1 = epi_dve(ic, 1, ps_g, ps_u)
                        pend_h[0] = (ic, 1, g1_1)
                    # smoothed butterfly emission (adaptive rate)
                    npop = 3 if len(pending) > 12 else 2
                    for _ in range(npop):
                        if pending:
                            butterfly(*pending.pop(0))
                # queue butterfly stages whose window closes at this chunk
                for s in range(1, L + 1):
                    span = 1 << s
                    if (ic + 1) % span == 0:
                        base = ic + 1 - span
                        hs = span // 2
                        for k in range(hs):
                            pending.append((s, base + k, base + k + hs))
            if pend_h[0] is not None:
                epi_h128(*pend_h[0])
                pend_h[0] = None

            # GEMM2 wd prefetch for the first pieces (DMA + ternarize run
            # during the quant tail; only 2 pieces fit the wdq rotation)
            def g2_piece(hc, piece, add_eng):
                wdq = wqp.tile([P, 8, P], BF16, tag="wq0", bufs=2,
                               name=f"wdq{hc}_{piece}")
                wt = wpipe.tile([P, 8, P], F32, tag="wf32",
                                name=f"wd{hc}_{piece}")
                c0 = piece * 8
                nc.sync.dma_start(
                    wt[:], wdt_v[:, c0:c0 + 8, hc * P:(hc + 1) * P])
                ternarize(wt[:].rearrange("p a b -> p (a b)"),
                          wdq[:].rearrange("p a b -> p (a b)"),
                          wsB[:, 2:3], add_eng)
                return wdq

            # flush remaining butterflies (the last-block cascade + stage 6
            # with fused absmax accumulation)
            while pending:
                butterfly(*pending.pop(0))

            wdq_pre = [g2_piece(0, 0, nc.gpsimd), g2_piece(0, 1, nc.gpsimd)]

            # ---------------- per-token scales for the intermediate quant
            sf = sc.tile([1, 1], F32, tag="sf")
            nc.vector.tensor_tensor(sf[:], wm_row[:, 2:3], wm_row[:, 1:2],
                                    OP.mult)
            nc.vector.tensor_scalar(sf[:], sf[:],
                                    inv_sqrt_i / (128.0 * 128.0 * 4.0),
                                    None, OP.mult)
            nc.vector.tensor_scalar(am_row[:], am_row[:], sf[:], None, OP.mult)

            rcs = []
            for tt in range(TTN):
                ts = slice(tt * NT, (tt + 1) * NT)
                nb = NT // P
                amT = sc.tile([P, nb], F32, tag="amT", name=f"amT_{tt}")
                for k in range(nb):
                    ptf = psp.tile([P, P], BF16, tag="tp", bufs=2,
                                   name=f"qpt{tt}_{k}")
                    nc.tensor.transpose(
                        ptf[:], m1[:, tt * NT + k * P:tt * NT + (k + 1) * P],
                        ident_b[:])
                    nc.vector.tensor_reduce(out=amT[:, k:k + 1], in_=ptf[:],
                                            op=OP.max, axis=AX)
                nc.vector.tensor_scalar(amT[:], amT[:], EPS, None, OP.max)
                sT = sc.tile([P, nb], F32, tag="sT", name=f"sT_{tt}")
                nc.vector.reciprocal(sT[:], amT[:])
                nc.vector.tensor_scalar(sT[:], sT[:], 128.0, None, OP.mult)
                srow = sc.tile([1, NT], F32, tag="srow", name=f"srow_{tt}")
                for k in range(nb):
                    cols = slice(tt * NT + k * P, tt * NT + (k + 1) * P)
                    prk = psp.tile([P, P], F32, tag="tp", bufs=2,
                                   name=f"prk{tt}_{k}")
                    nc.tensor.transpose(prk[:1, :], amT[:, k:k + 1],
                                        ident_f[:])
                    nc.vector.tensor_tensor(am_row[:, cols], am_row[:, cols],
                                            prk[:1, :], OP.mult)
                    psk = psp.tile([P, P], F32, tag="tp", bufs=2,
                                   name=f"psk{tt}_{k}")
                    nc.tensor.transpose(psk[:1, :], sT[:, k:k + 1],
                                        ident_f[:])
                    nc.scalar.copy(srow[:, k * P:(k + 1) * P], psk[:1, :])
                psb = psp.tile([P, NT], F32, tag="mm", bufs=6,
                               name=f"psb_{tt}")
                nc.tensor.matmul(psb[:], ones1[:], srow[:],
                                 start=True, stop=True)
                rc = s2k.tile([P, NT], F32, tag="g1", bufs=2,
                              name=f"rc_{tt}")
                nc.scalar.copy(rc[:], psb[:])
                rcs.append(rc)
                # refresh the per-token output-dequant broadcast for GEMM2
                nc.gpsimd.partition_broadcast(bcast[:, ts], am_row[:, ts])

            # ---------------- intermediate act-quant: exact f32 rint via
            # PSUM staging, chunk-major so GEMM2 pipelines right behind it.
            # hc=0's remaining wd pieces are emitted inside the wavefront so
            # their ternarize tracks the quant progress.
            for c in range(C):
                pc = loc[c]
                for tt in range(TTN):
                    ts = slice(tt * NT, (tt + 1) * NT)
                    psq = psp.tile([P, NT], F32, tag="mm", bufs=6,
                                   name=f"q{c}_{tt}")
                    nc.vector.tensor_tensor(psq[:], interm[:, pc, ts],
                                              rcs[tt][:], OP.mult)
                    nc.vector.tensor_scalar(psq[:], psq[:], QCLIP, MAGIC,
                                            OP.min, OP.add)
                    nc.scalar.activation(interm[:, pc, ts], psq[:],
                                         ACT_FN.Identity, bias=nmagicB[:])
                if (c + 1) % 8 == 0 and 2 <= (c + 1) // 8 <= 7:
                    wdq_pre.append(g2_piece(0, (c + 1) // 8, nc.gpsimd))

            # ---------------- GEMM2: hc-outer, both token halves interleaved
            # (wd DMA'd + ternarized exactly once)
            for hc in range(HC):
                pso = [psp.tile([P, NT], F32, tag="mm", bufs=6,
                                name=f"pso{hc}_{t}") for t in range(TTN)]
                for piece in range(8):
                    if hc == 0:
                        wdq = wdq_pre[piece]
                    else:
                        wdq = g2_piece(hc, piece, nc.vector)
                    for j in range(8):
                        c = piece * 8 + j
                        for tt in range(TTN):
                            nc.tensor.matmul(
                                pso[tt][:], wdq[:, j, :],
                                interm[:, loc[c], tt * NT:(tt + 1) * NT],
                                start=(c == 0), stop=(c == C - 1),
                                skip_group_check=True)
                for tt in range(TTN):
                    o1 = s2k.tile([P, NT], F32, tag="g1", bufs=2,
                                  name=f"o{hc}_{tt}")
                    nc.vector.tensor_tensor(o1[:], pso[tt][:],
                                            bcast[:, tt * NT:(tt + 1) * NT],
                                            OP.mult)
                    for k in range(NT // P):
                        tb = tt * (NT // P) + k
                        po = psp.tile([P, P], F32, tag="tp", bufs=2,
                                      name=f"po{hc}_{tb}")
                        nc.tensor.transpose(po[:], o1[:, k * P:(k + 1) * P],
                                            ident_f[:])
                        ot = s2k.tile([P, P], F32, tag="ot", bufs=2,
                                      name=f"ot{hc}_{tb}")
                        nc.scalar.copy(ot[:], po[:])
                        nc.sync.dma_start(
                            out_d.ap()[tb * P:(tb + 1) * P,
                                       hc * P:(hc + 1) * P], ot[:])

    nc.compile()
    return nc


_PROG_CACHE = {}
_LAST_IN_MAPS = None


def kernel(x, w_gate, w_up, w_down):
    from concourse.bass_utils import run_bass_kernel_spmd

    B, S, H = x.shape
    I = w_gate.shape[0]
    n_cores = 8
    M = B * S
    T = M // n_cores

    key = (T, H, I, n_cores)
    if key not in _PROG_CACHE:
        _PROG_CACHE[key] = build_program(T, H, I, n_cores)
    nc = _PROG_CACHE[key]

    xf = np.ascontiguousarray(x.reshape(M, H).astype(np.float32))
    wgT = np.ascontiguousarray(w_gate.T)     # [H, I]
    wuT = np.ascontiguousarray(w_up.T)       # [H, I]
    wdT = np.ascontiguousarray(w_down.T)     # [I, H]
    hm = hadamard128()
    HS, IS = H // n_cores, I // n_cores
    in_maps = []
    for c in range(n_cores):
        in_maps.append({
            "x_s": xf[c * T:(c + 1) * T],
            "wgt": wgT, "wut": wuT, "wdt": wdT,
            "wg_s": np.ascontiguousarray(wgT[c * HS:(c + 1) * HS]),
            "wu_s": np.ascontiguousarray(wuT[c * HS:(c + 1) * HS]),
            "wd_s": np.ascontiguousarray(wdT[c * IS:(c + 1) * IS]),
            "hmat": hm,
        })
    global _LAST_IN_MAPS
    _LAST_IN_MAPS = in_maps
    res = run_bass_kernel_spmd(nc, in_maps, list(range(n_cores)))
    out = np.concatenate([res.results[c]["out_s"] for c in range(n_cores)], 0)
    return out.reshape(B, S, H).astype(np.float32)


# revision 19
# speedup vs baseline: 1.0481x; 1.0481x over previous
"""BitNet MLP (ternary gate/up GEMM + silu*up + Hadamard + act-quant + down GEMM)
on 8 Trainium2 NeuronCores — v6.

Token-data-parallel (T=1024 tokens/core). v4/v5 -> v6 (trace-driven):
  - the weight-scale AllReduce is split into three tiny collectives issued
    as each shard sum completes, so the first ternarize waits only on the
    gate collective (~50us earlier GEMM1 start),
  - PSUM workspace uses 2-bank [P,2,512] tiles: GEMM1 (gate|up) pairs and
    (H128 tt0|tt1) pairs, full-chunk act-quant ops and GEMM2 (tt0|tt1)
    accumulators — halves the per-op PSUM access overhead (~450ns/op),
  - intermediate act-quant runs at full token width per chunk (one
    mult + one min/MAGIC + one ACT per 128x1024 chunk), chunk-major so
    GEMM2's first h-block pipelines right behind the wavefront,
  - per-token absmax: |chunk| on the ACT engine + a single DVE max chain,
  - exact butterflies via spare-slot ping-pong (v4), H128 on float32r with
    the tt1 matmul deferred into the next chunk's PE stream.
"""

import sys

sys.path.insert(0, "/opt/trn_rl_repo")

import numpy as np

import concourse.bass as bass
import concourse.mybir as mybir
import concourse.tile as tile
from concourse import bacc, bass_isa

F32 = mybir.dt.float32
F32R = mybir.dt.float32r
BF16 = mybir.dt.bfloat16
AX = mybir.AxisListType.X
OP = mybir.AluOpType
ACT_FN = mybir.ActivationFunctionType

MAGIC = 12582912.0  # 1.5*2^23: (x + MAGIC) - MAGIC == rint(x) in f32
EPS = 1e-5
QCLIP = 127.4375


def hadamard128():
    h = np.array([[1.0]], dtype=np.float32)
    while h.shape[0] < 128:
        h = np.block([[h, h], [h, -h]]).astype(np.float32)
    return h


def build_program(T, H, I, n_cores):
    from concourse.masks import make_identity

    P = 128
    HC = H // P                    # 16 h-chunks
    C = I // P                     # 64 i-chunks (fwht H64 factor)
    NT = min(512, T)
    TTN = T // NT                  # 2 token halves
    n_tb = T // P                  # 8 token 128-blocks
    L = int(np.log2(C))            # 6 butterfly stages over the chunk index
    assert 2 ** L == C and T % P == 0 and H % 1024 == 0 and I % P == 0
    inv_sqrt_i = float(1.0 / np.sqrt(I))
    wcount = float(I) * float(H)

    nc = bacc.Bacc("TRN2", target_bir_lowering=False, num_devices=n_cores)

    x_d = nc.dram_tensor("x_s", [T, H], F32, kind="ExternalInput")
    wgt_d = nc.dram_tensor("wgt", [H, I], F32, kind="ExternalInput")
    wut_d = nc.dram_tensor("wut", [H, I], F32, kind="ExternalInput")
    wdt_d = nc.dram_tensor("wdt", [I, H], F32, kind="ExternalInput")
    wgs_d = nc.dram_tensor("wg_s", [H // n_cores, I], F32, kind="ExternalInput")
    wus_d = nc.dram_tensor("wu_s", [H // n_cores, I], F32, kind="ExternalInput")
    wds_d = nc.dram_tensor("wd_s", [I // n_cores, H], F32, kind="ExternalInput")
    hm_d = nc.dram_tensor("hmat", [P, P], F32, kind="ExternalInput")
    out_d = nc.dram_tensor("out_s", [T, H], F32, kind="ExternalOutput")

    cc_in = nc.dram_tensor("cc_in", [1, 4], F32)
    cc_out = nc.dram_tensor("cc_out", [1, 4], F32, addr_space="Shared")

    # DRAM views for transposed-tile loads: [p, q, i] with h = q*128 + p
    wgt_v = wgt_d.ap().rearrange("(q p) i -> p q i", p=P)
    wut_v = wut_d.ap().rearrange("(q p) i -> p q i", p=P)
    wdt_v = wdt_d.ap().rearrange("(c p) h -> p c h", p=P)

    with tile.TileContext(nc) as tc:
        with (
            tc.tile_pool(name="consts", bufs=1) as consts,
            tc.tile_pool(name="wpipe", bufs=2) as wpipe,    # f32 stage [P,8,128]
            tc.tile_pool(name="wqp", bufs=1) as wqp,        # ternarized weights
            tc.tile_pool(name="xq", bufs=1) as xqp,         # xqT persistent
            tc.tile_pool(name="ip", bufs=1) as ip,          # interm
            tc.tile_pool(name="s2k", bufs=1) as s2k,        # scratch
            tc.tile_pool(name="sc", bufs=1) as sc,          # scales/rows
            tc.tile_pool(name="ps", bufs=1, space="PSUM") as psp,
        ):
            # ---------------- constants
            hmtmp = wpipe.tile([P, 8, P], F32, tag="wf32", name="hmtmp")
            nc.sync.dma_start(hmtmp[:, 0, :], hm_d.ap())
            ident_f = consts.tile([P, P], F32, tag="ident_f")
            make_identity(nc, ident_f[:])
            ident_b = consts.tile([P, P], BF16, tag="ident_b")
            nc.vector.tensor_copy(ident_b[:], ident_f[:])
            hmat_r = consts.tile([P, P], F32R, tag="hmat_r")
            nc.vector.tensor_copy(hmat_r[:], hmtmp[:, 0, :])
            magicB = consts.tile([P, 1], F32, tag="magicB")
            nc.vector.memset(magicB[:], MAGIC)
            nmagicB = consts.tile([P, 1], F32, tag="nmagicB")
            nc.vector.memset(nmagicB[:], -MAGIC)
            ones1 = consts.tile([1, P], F32, tag="ones1")
            nc.vector.memset(ones1[:], 1.0)
            halfn = consts.tile([P, 1], F32, tag="halfn")
            nc.vector.memset(halfn[:], -0.5)
            halfp = consts.tile([P, 1], F32, tag="halfp")
            nc.vector.memset(halfp[:], 0.5)
            m1 = sc.tile([P, T], BF16, tag="m1")
            nc.vector.memset(m1[:], 0.0)

            # ---------------- weight-scale pass: per-matrix shard |w| sum +
            # its own tiny AllReduce, issued as soon as the sum is ready so
            # the gate collective (which gates GEMM1) completes first.
            wm_row = sc.tile([1, 4], F32, tag="wm_row")
            ws_row = sc.tile([1, 4], F32, tag="ws_row")
            wsB = sc.tile([P, 4], F32, tag="wsB")

            def shard_scale(src_d, rows, cols, k, tag):
                ntr, ntc = rows // P, cols // 1024
                acc = sc.tile([P, ntr * ntc], F32, tag=f"acc_{tag}")
                for r in range(ntr):
                    for q in range(ntc):
                        t = wpipe.tile([P, 8, P], F32, tag="wf32",
                                       name=f"as_{tag}_{r}_{q}")
                        nc.sync.dma_start(
                            t[:].rearrange("p a b -> p (a b)"),
                            src_d.ap()[r * P:(r + 1) * P,
                                       q * 1024:(q + 1) * 1024])
                        nc.vector.tensor_reduce(
                            out=acc[:, r * ntc + q:r * ntc + q + 1],
                            in_=t[:].rearrange("p a b -> p (a b)"),
                            op=OP.add, axis=AX, apply_absolute_value=True)
                tot = sc.tile([P, 1], F32, tag=f"tot_{tag}")
                nc.vector.tensor_reduce(out=tot[:], in_=acc[:], op=OP.add,
                                        axis=AX)
                red = sc.tile([P, 1], F32, tag=f"red_{tag}")
                nc.gpsimd.partition_all_reduce(
                    red[:], tot[:], channels=P, reduce_op=bass_isa.ReduceOp.add)
                ccin_sb = sc.tile([1, 1], F32, tag=f"ccin_{tag}")
                nc.vector.tensor_copy(ccin_sb[:], red[0:1, :])
                nc.sync.dma_start(cc_in.ap()[:, k:k + 1], ccin_sb[:])
                nc.gpsimd.collective_compute(
                    "AllReduce", OP.add, ins=[cc_in.ap()[:, k:k + 1]],
                    outs=[cc_out.ap()[:, k:k + 1]],
                    replica_groups=[list(range(n_cores))])
                sum_sb = sc.tile([1, 1], F32, tag=f"sum_{tag}")
                nc.sync.dma_start(sum_sb[:], cc_out.ap()[:, k:k + 1])
                # finalize this matrix's scale
                nc.vector.tensor_scalar(wm_row[:, k:k + 1], sum_sb[:],
                                        1.0 / wcount, EPS, OP.mult, OP.max)
                nc.vector.reciprocal(ws_row[:, k:k + 1], wm_row[:, k:k + 1])
                nc.gpsimd.partition_broadcast(wsB[:, k:k + 1],
                                              ws_row[:, k:k + 1])

            shard_scale(wgs_d, H // n_cores, I, 0, "g")
            shard_scale(wus_d, H // n_cores, I, 1, "u")
            shard_scale(wds_d, I // n_cores, H, 2, "d")

            # ---------------- x: act-quant + transpose into xqT (bf16 ints)
            xqt = xqp.tile([P, HC, T], BF16, tag="xqt")
            am_row = sc.tile([1, T], F32, tag="am_row")
            for tb in range(n_tb):
                xt = [wpipe.tile([P, 8, P], F32, tag="wf32", name=f"x{tb}_{q}")
                      for q in range(2)]
                for q in range(2):
                    nc.sync.dma_start(
                        xt[q][:].rearrange("p a b -> p (a b)"),
                        x_d.ap()[tb * P:(tb + 1) * P,
                                 q * 1024:(q + 1) * 1024])
                am2 = sc.tile([P, 2], F32, tag="am2")
                for q in range(2):
                    nc.vector.tensor_reduce(
                        out=am2[:, q:q + 1],
                        in_=xt[q][:].rearrange("p a b -> p (a b)"),
                        op=OP.max, axis=AX, apply_absolute_value=True)
                amc = sc.tile([P, 1], F32, tag="amc")
                nc.vector.tensor_reduce(out=amc[:], in_=am2[:], op=OP.max,
                                        axis=AX)
                nc.vector.tensor_scalar(amc[:], amc[:], EPS, None, OP.max)
                sx = sc.tile([P, 1], F32, tag="sx")
                nc.vector.reciprocal(sx[:], amc[:])
                nc.vector.tensor_scalar(sx[:], sx[:], 128.0, None, OP.mult)
                for q in range(2):
                    f = xt[q][:].rearrange("p a b -> p (a b)")
                    nc.scalar.activation(f, f, ACT_FN.Identity,
                                         bias=magicB[:], scale=sx[:])
                    xqb = s2k.tile([P, 1024], BF16, tag="ta",
                                   name=f"xqb{tb}_{q}")
                    nc.vector.tensor_scalar(xqb[:], f, -MAGIC, 127.0,
                                            OP.add, OP.min)
                    for k in range(8):
                        hc = q * 8 + k
                        pt = psp.tile([P, P], BF16, tag="tp", bufs=2,
                                      name=f"pt{tb}_{hc}")
                        nc.tensor.transpose(pt[:], xqb[:, k * P:(k + 1) * P],
                                            ident_b[:])
                        nc.scalar.copy(xqt[:, hc, tb * P:(tb + 1) * P], pt[:])
                pr = psp.tile([P, P], F32, tag="tp", bufs=2, name=f"pr{tb}")
                nc.tensor.transpose(pr[:1, :], amc[:], ident_f[:])
                nc.scalar.copy(am_row[:, tb * P:(tb + 1) * P], pr[:1, :])

            # bcast = per-token gate dequant (am * wm_g / 256), all partitions
            sg = sc.tile([1, 1], F32, tag="sg")
            nc.vector.tensor_scalar(sg[:], wm_row[:, 0:1], 1.0 / 256.0, None,
                                    OP.mult)
            sgB = sc.tile([P, 1], F32, tag="sgB")
            nc.gpsimd.partition_broadcast(sgB[:], sg[:])
            bcast = sc.tile([P, T], F32, tag="bcast")
            nc.gpsimd.partition_broadcast(bcast[:], am_row[:])
            nc.vector.tensor_scalar(bcast[:], bcast[:], sgB[:], None, OP.mult)

            # ---------------- ternarize helper: f32 tile -> bf16 2*{-1,0,1}
            # 2*clip(rint(w*ws),-1,1) = Sign(w*ws-0.5) + Sign(w*ws+0.5)
            _tz = [0]
            def ternarize(wt_f, dst_b, ws_ap, add_eng):
                _tz[0] += 1
                ta = s2k.tile([P, 1024], BF16, tag="ta", name=f"ta{_tz[0]}")
                nc.scalar.activation(dst_b, wt_f, ACT_FN.Sign,
                                     bias=halfn[:], scale=ws_ap)
                nc.scalar.activation(ta[:], wt_f, ACT_FN.Sign,
                                     bias=halfp[:], scale=ws_ap)
                add_eng.tensor_tensor(dst_b, dst_b, ta[:], OP.add)

            # ---------------- in-flight H64 butterfly, bit-exact via a
            # spare physical chunk slot: s = a+b -> spare ; b = a-b in place;
            # the old a-slot becomes the new spare.
            interm = ip.tile([P, C + 1, T], BF16, tag="interm")
            loc = list(range(C))
            spare = [C]
            _ab = [0]

            def butterfly(stage, a_idx, b_idx):
                pa, pb, sp = loc[a_idx], loc[b_idx], spare[0]
                A = interm[:, pa, :]
                B = interm[:, pb, :]
                S_ = interm[:, sp, :]
                nc.vector.tensor_tensor(S_, A, B, OP.add)
                nc.vector.tensor_tensor(B, A, B, OP.subtract)
                loc[a_idx] = sp
                spare[0] = pa
                if stage == L:
                    # accumulate per-(partition,token) absmax: |chunk| on the
                    # (otherwise idle) ACT engine, single max chain on DVE
                    for pidx in (loc[a_idx], loc[b_idx]):
                        _ab[0] += 1
                        ab = s2k.tile([P, T], BF16, tag="ab", bufs=1,
                                      name=f"ab{_ab[0]}")
                        nc.scalar.activation(ab[:], interm[:, pidx, :],
                                             ACT_FN.Abs)
                        nc.vector.tensor_tensor(m1[:], m1[:], ab[:], OP.max)

            pending = []  # butterfly pairs not yet emitted (burst smoothing)

            # ---------------- GEMM1 + epilogue (H128 fused) -> interm bf16
            def epi_dve(ic, tt, ps_gu):
                ts = slice(tt * NT, (tt + 1) * NT)
                g1 = s2k.tile([P, NT], F32R, tag="g1", bufs=2,
                              name=f"g1_{ic}{tt}")
                nc.vector.tensor_tensor(g1[:], ps_gu[:, 0, :], bcast[:, ts],
                                        OP.mult)
                nc.scalar.activation(g1[:], g1[:], ACT_FN.Silu)
                nc.vector.tensor_tensor(g1[:], g1[:], ps_gu[:, 1, :], OP.mult)
                return g1

            def epi_h128(ic, tt, g1, psH2):
                ts = slice(tt * NT, (tt + 1) * NT)
                nc.tensor.matmul(psH2[:, tt, :], hmat_r[:], g1[:],
                                 start=True, stop=True)
                nc.scalar.copy(interm[:, ic, ts], psH2[:, tt, :])

            pend_h = [None]
            for ic in range(C):
                wq = []
                for mi, w_v in ((0, wgt_v), (1, wut_v)):
                    w = wqp.tile([P, HC, P], BF16, tag=f"wq{mi}",
                                 bufs=(2 if mi == 0 else 1),
                                 name=f"wq{mi}_{ic}")
                    wq.append(w)
                    for half in range(2):
                        wt = wpipe.tile([P, 8, P], F32, tag="wf32",
                                        name=f"w{ic}_{mi}_{half}")
                        nc.sync.dma_start(
                            wt[:],
                            w_v[:, half * 8:(half + 1) * 8,
                                ic * P:(ic + 1) * P])
                        ternarize(
                            wt[:].rearrange("p a b -> p (a b)"),
                            w[:, half * 8:(half + 1) * 8, :].rearrange(
                                "p a b -> p (a b)"),
                            wsB[:, mi:mi + 1], nc.vector)
                for tt in range(TTN):
                    ts = slice(tt * NT, (tt + 1) * NT)
                    ps_gu = psp.tile([P, 2, NT], F32, tag="mm2", bufs=3,
                                     name=f"psgu{ic}_{tt}")
                    for hc in range(HC):
                        nc.tensor.matmul(ps_gu[:, 0, :], wq[0][:, hc, :],
                                         xqt[:, hc, ts],
                                         start=(hc == 0), stop=(hc == HC - 1))
                    for hc in range(HC):
                        nc.tensor.matmul(ps_gu[:, 1, :], wq[1][:, hc, :],
                                         xqt[:, hc, ts],
                                         start=(hc == 0), stop=(hc == HC - 1))
                    if tt == 0:
                        # H128 of the previous chunk's tt1 (deferred: its
                        # product is ready by now; 2 mm groups of slack)
                        if pend_h[0] is not None:
                            epi_h128(*pend_h[0])
                            pend_h[0] = None
                        g1_0 = epi_dve(ic, 0, ps_gu)
                    else:
                        psH2 = psp.tile([P, 2, NT], F32, tag="mm2", bufs=3,
                                        name=f"psh{ic}")
                        epi_h128(ic, 0, g1_0, psH2)
                        g1_1 = epi_dve(ic, 1, ps_gu)
                        pend_h[0] = (ic, 1, g1_1, psH2)
                    # smoothed butterfly emission (adaptive rate)
                    npop = 3 if len(pending) > 12 else 2
                    for _ in range(npop):
                        if pending:
                            butterfly(*pending.pop(0))
                # queue butterfly stages whose window closes at this chunk
                for s in range(1, L + 1):
                    span = 1 << s
                    if (ic + 1) % span == 0:
                        base = ic + 1 - span
                        hs = span // 2
                        for k in range(hs):
                            pending.append((s, base + k, base + k + hs))
            if pend_h[0] is not None:
                epi_h128(*pend_h[0])
                pend_h[0] = None

            # GEMM2 wd prefetch helper (DMA + ternarize; the first pieces run
            # during the quant tail)
            def g2_piece(hc, piece, add_eng):
                wdq = wqp.tile([P, 8, P], BF16, tag="wq0", bufs=2,
                               name=f"wdq{hc}_{piece}")
                wt = wpipe.tile([P, 8, P], F32, tag="wf32",
                                name=f"wd{hc}_{piece}")
                c0 = piece * 8
                nc.sync.dma_start(
                    wt[:], wdt_v[:, c0:c0 + 8, hc * P:(hc + 1) * P])
                ternarize(wt[:].rearrange("p a b -> p (a b)"),
                          wdq[:].rearrange("p a b -> p (a b)"),
                          wsB[:, 2:3], add_eng)
                return wdq

            # flush remaining butterflies (the last-block cascade + stage 6
            # with fused absmax accumulation)
            while pending:
                butterfly(*pending.pop(0))

            wdq_pre = [g2_piece(0, 0, nc.gpsimd), g2_piece(0, 1, nc.gpsimd)]

            # ---------------- per-token scales for the intermediate quant
            sf = sc.tile([1, 1], F32, tag="sf")
            nc.vector.tensor_tensor(sf[:], wm_row[:, 2:3], wm_row[:, 1:2],
                                    OP.mult)
            nc.vector.tensor_scalar(sf[:], sf[:],
                                    inv_sqrt_i / (128.0 * 128.0 * 4.0),
                                    None, OP.mult)
            nc.vector.tensor_scalar(am_row[:], am_row[:], sf[:], None, OP.mult)

            psb = psp.tile([P, 2, NT], F32, tag="mm2", bufs=3, name="psb")
            rcT = sc.tile([P, T], F32, tag="rcT")
            for tt in range(TTN):
                ts = slice(tt * NT, (tt + 1) * NT)
                nb = NT // P
                amT = sc.tile([P, nb], F32, tag="amT", name=f"amT_{tt}")
                for k in range(nb):
                    ptf = psp.tile([P, P], BF16, tag="tp", bufs=2,
                                   name=f"qpt{tt}_{k}")
                    nc.tensor.transpose(
                        ptf[:], m1[:, tt * NT + k * P:tt * NT + (k + 1) * P],
                        ident_b[:])
                    nc.vector.tensor_reduce(out=amT[:, k:k + 1], in_=ptf[:],
                                            op=OP.max, axis=AX)
                nc.vector.tensor_scalar(amT[:], amT[:], EPS, None, OP.max)
                sT = sc.tile([P, nb], F32, tag="sT", name=f"sT_{tt}")
                nc.vector.reciprocal(sT[:], amT[:])
                nc.vector.tensor_scalar(sT[:], sT[:], 128.0, None, OP.mult)
                srow = rcT[0:1, ts]
                for k in range(nb):
                    cols = slice(tt * NT + k * P, tt * NT + (k + 1) * P)
                    prk = psp.tile([P, P], F32, tag="tp", bufs=2,
                                   name=f"prk{tt}_{k}")
                    nc.tensor.transpose(prk[:1, :], amT[:, k:k + 1],
                                        ident_f[:])
                    nc.vector.tensor_tensor(am_row[:, cols], am_row[:, cols],
                                            prk[:1, :], OP.mult)
                    psk = psp.tile([P, P], F32, tag="tp", bufs=2,
                                   name=f"psk{tt}_{k}")
                    nc.tensor.transpose(psk[:1, :], sT[:, k:k + 1],
                                        ident_f[:])
                    nc.scalar.copy(srow[:, k * P:(k + 1) * P], psk[:1, :])
                nc.tensor.matmul(psb[:, tt, :], ones1[:], srow,
                                 start=True, stop=True)
                # refresh the per-token output-dequant broadcast for GEMM2
                nc.gpsimd.partition_broadcast(bcast[:, ts], am_row[:, ts])
            nc.scalar.copy(rcT[:], psb[:].rearrange("p a b -> p (a b)"))

            # ---------------- intermediate act-quant: exact f32 rint via
            # PSUM staging, full token width per chunk, chunk-major so
            # GEMM2's hc=0 pipelines right behind the wavefront. hc=0's
            # remaining wd pieces are emitted inside the wavefront.
            for c in range(C):
                pc = loc[c]
                psq = psp.tile([P, 2, NT], F32, tag="mm2", bufs=3,
                               name=f"q{c}")
                qf = psq[:].rearrange("p a b -> p (a b)")
                nc.vector.tensor_tensor(qf, interm[:, pc, :], rcT[:], OP.mult)
                nc.vector.tensor_scalar(qf, qf, QCLIP, MAGIC, OP.min, OP.add)
                nc.scalar.activation(interm[:, pc, :], qf, ACT_FN.Identity,
                                     bias=nmagicB[:])
                if (c + 1) % 8 == 0 and 2 <= (c + 1) // 8 <= 7:
                    wdq_pre.append(g2_piece(0, (c + 1) // 8, nc.gpsimd))

            # ---------------- GEMM2: hc-outer, both token halves interleaved
            # (wd DMA'd + ternarized exactly once)
            for hc in range(HC):
                pso = psp.tile([P, 2, NT], F32, tag="mm2", bufs=3,
                               name=f"pso{hc}")
                for piece in range(8):
                    if hc == 0:
                        wdq = wdq_pre[piece]
                    else:
                        wdq = g2_piece(hc, piece, nc.vector)
                    for j in range(8):
                        c = piece * 8 + j
                        for tt in range(TTN):
                            nc.tensor.matmul(
                                pso[:, tt, :], wdq[:, j, :],
                                interm[:, loc[c], tt * NT:(tt + 1) * NT],
                                start=(c == 0), stop=(c == C - 1),
                                skip_group_check=True)
                for tt in range(TTN):
                    o1 = s2k.tile([P, NT], F32, tag="g1", bufs=2,
                                  name=f"o{hc}_{tt}")
                    nc.vector.tensor_tensor(o1[:], pso[:, tt, :],
                                            bcast[:, tt * NT:(tt + 1) * NT],
                                            OP.mult)
                    for k in range(NT // P):
                        tb = tt * (NT // P) + k
                        po = psp.tile([P, P], F32, tag="tp", bufs=2,
                                      name=f"po{hc}_{tb}")
                        nc.tensor.transpose(po[:], o1[:, k * P:(k + 1) * P],
                                            ident_f[:])
                        ot = s2k.tile([P, P], F32, tag="ot", bufs=2,
                                      name=f"ot{hc}_{tb}")
                        nc.scalar.copy(ot[:], po[:])
                        nc.sync.dma_start(
                            out_d.ap()[tb * P:(tb + 1) * P,
                                       hc * P:(hc + 1) * P], ot[:])

    nc.compile()
    return nc


_PROG_CACHE = {}
_LAST_IN_MAPS = None


def kernel(x, w_gate, w_up, w_down):
    from concourse.bass_utils import run_bass_kernel_spmd

    B, S, H = x.shape
    I = w_gate.shape[0]
    n_cores = 8
    M = B * S
    T = M // n_cores

    key = (T, H, I, n_cores)
    if key not in _PROG_CACHE:
        _PROG_CACHE[key] = build_program(T, H, I, n_cores)
    nc = _PROG_CACHE[key]

    xf = np.ascontiguousarray(x.reshape(M, H).astype(np.float32))
    wgT = np.ascontiguousarray(w_gate.T)     # [H, I]
    wuT = np.ascontiguousarray(w_up.T)       # [H, I]
    wdT = np.ascontiguousarray(w_down.T)     # [I, H]
    hm = hadamard128()
    HS, IS = H // n_cores, I // n_cores
    in_maps = []
    for c in range(n_cores):
        in_maps.append({
            "x_s": xf[c * T:(c + 1) * T],
            "wgt": wgT, "wut": wuT, "wdt": wdT,
            "wg_s": np.ascontiguousarray(wgT[c * HS:(c + 1) * HS]),
            "wu_s": np.ascontiguousarray(wuT[c * HS:(c + 1) * HS]),
            "wd_s": np.ascontiguousarray(wdT[c * IS:(c + 1) * IS]),
            "hmat": hm,
        })
    global _LAST_IN_MAPS
    _LAST_IN_MAPS = in_maps
    res = run_bass_kernel_spmd(nc, in_maps, list(range(n_cores)))
    out = np.concatenate([res.results[c]["out_s"] for c in range(n_cores)], 0)
    return out.reshape(B, S, H).astype(np.float32)
